# revision 48
# baseline (speedup 1.0000x reference)
"""Trainium2 Bass kernel for the MultiHeadAttention transformer block.

Sharding: 8 cores, core c handles batch b=c//2 and query-row half
(c%2)*1024 .. +1024, all 8 heads.  Each core is fully independent
(no collectives).

Key optimizations vs the dense baseline:
  - the attention mask zeroes ~half the keys, so the host packs the
    unmasked key rows per (batch, head) before upload.  Packed K/V
    (padded to whole 128-key tiles) shrink the QK matmuls, the
    softmax Exp (the ScalarE bottleneck: exp is ACT-only) and the AV
    matmuls by ~1.8x.  Padding is exact with no mask bias: pad K
    columns are zero so pad scores exp to 1, but pad V rows and pad
    slots of the denominator ones-column are zero, so pads contribute
    nothing to numerator or denominator.
  - the attention inner loop is software-pipelined so the ScalarE Exp
    stream rarely waits: QK(kt+1) is emitted before AV(kt), and the
    projection work for the next head pair is sprinkled between the
    kt steps to keep the PE warm while ACT runs.  (K8/V8 can switch
    the K/V paths to fp8e4 -- measured ~1e-2 rel err and no LDW win
    on this stack, so both stay bf16.)

Layout: everything transposed on chip -- [channel on partitions,
sequence on free dim]:
  - K projection col-tiled (even head -> PSUM partitions 0:64, odd ->
    64:128, concurrent);  QK row-tiled (the two 64-channel
    contractions use PE rows 0:64 / 64:128 concurrently);  AV
    col-tiled (two heads' dv-channels -> one PSUM bank, the exact
    xbf channel layout).  Softmax denominators accumulate in one PSUM
    bank via 1-column ones matmuls col-tiled at array columns
    0/32/64/96, and are applied via a DRAM-bounce partition-broadcast
    DMA followed by an on-chip reciprocal.  (Caution: a DMA must
    never read the output of a custom-DVE op like
    reciprocal_approx_fast -- the framework does not order it.)
  - LayerNorm mean/var come from ones-column matmuls over the channel
    (partition) dim; LN0 stats accumulate during attention as head
    pairs complete.  The LN0 -> fc -> LN1 tail runs in two
    512-query halves with rank-1 matmul broadcasts for mu/rstd.  The
    fc consumes LN0^T and produces out^T, un-transposed by the host
    for free.
"""

import math
import sys

if "/opt/trn_rl_repo" not in sys.path:
    sys.path.insert(0, "/opt/trn_rl_repo")

import numpy as np

import concourse.bacc as bacc
import concourse.bass as bass
import concourse.bass_isa as bass_isa
import concourse.tile as tile
from concourse import mybir
from concourse.bass_utils import run_bass_kernel_spmd

H, D, DK, DV = 8, 512, 64, 64
B, L = 4, 2048
P = 128
LQ = L // 2          # query rows per core
NCORES = 8
EPS = 1e-5
F32 = mybir.dt.float32
BF16 = mybir.dt.bfloat16
K8 = mybir.dt.bfloat16
V8 = mybir.dt.bfloat16
AF = mybir.ActivationFunctionType
Alu = mybir.AluOpType

DT = D // P     # 4 d-tiles
NB = LQ // 512  # 2 psum-bank columns of queries

_CACHE = {}


def _emit(nc, tc, kts):
    """kts: tuple of 4 ints, key tiles (of 128) per head pair."""
    ktmax = max(kts)
    nkmax = ktmax * P

    qT = nc.dram_tensor("qT", [P, DT, LQ], BF16, kind="ExternalInput")
    kTp = nc.dram_tensor("kTp", [H, P, DT, nkmax], K8, kind="ExternalInput")
    vTp = nc.dram_tensor("vTp", [H, P, DT, nkmax], V8, kind="ExternalInput")
    WqT = nc.dram_tensor("WqT", [P, DT, D], BF16, kind="ExternalInput")
    WkT = nc.dram_tensor("WkT", [P, DT, D], K8, kind="ExternalInput")
    WvT = nc.dram_tensor("WvT", [P, DT, D], BF16, kind="ExternalInput")
    fcwT = nc.dram_tensor("fcwT", [P, DT, D], BF16, kind="ExternalInput")
    onescol = nc.dram_tensor("onescol", [P, H * ktmax], BF16,
                             kind="ExternalInput")
    vecs = nc.dram_tensor("vecs", [P, 5, DT], F32, kind="ExternalInput")
    out = nc.dram_tensor("out", [P, DT, LQ], BF16, kind="ExternalOutput")

    with (
        tc.tile_pool(name="consts", bufs=1) as consts,
        tc.tile_pool(name="projout", bufs=1) as projout,
        tc.tile_pool(name="dramp", bufs=4, space="DRAM") as dramp,
        tc.tile_pool(name="ps", bufs=1, space="PSUM") as ps,
    ):
        # ---- constants resident for the whole kernel ----
        gbT = consts.tile([P, 5, DT], F32)   # g0,b0,g1,b1,fcb as [p, dt]
        nc.sync.dma_start(out=gbT, in_=vecs[:, :, :])
        eps_t = consts.tile([P, 1], F32)
        nc.vector.memset(eps_t, EPS)
        ones_st = consts.tile([P, 1], BF16)  # stats reduction column
        nc.vector.memset(ones_st, 1.0)
        ones_r1 = consts.tile([1, P], BF16)  # rank-1 broadcast row
        nc.vector.memset(ones_r1, 1.0)
        warm = consts.tile([P, 512], BF16)   # PE warm-up fodder
        nc.vector.memset(warm[:, :], 0.0)
        oc_s = consts.tile([P, H * ktmax], BF16)  # pad-aware ones column
        nc.sync.dma_start(out=oc_s, in_=onescol[:, :])
        expw = consts.tile([1, 1], F32)      # preload the Exp table set
        nc.scalar.activation(out=expw, in_=eps_t[0:1, 0:1], func=AF.Exp,
                             bias=eps_t[0:1, :], scale=1.0)

        # ---- persistent big tiles ----
        QT_s = projout.tile([P, DT, LQ], BF16)          # Q^T  [ch, lq]
        KT_s = projout.tile([P, H // 2, nkmax], BF16)   # per-pair K^T packed
        V_s = projout.tile([P, H, ktmax, DV + 1], BF16)  # V rows + ones col
        qT_s = projout.tile([P, DT, LQ], BF16)          # also the residual

        # scatter the pad-aware ones column into V_s
        nc.vector.tensor_copy(
            V_s[:, :, :, DV],
            oc_s.rearrange("p (h k) -> p h k", h=H))

        nc.sync.dma_start(out=qT_s, in_=qT[:, :, :])

        with tc.tile_pool(name="statin", bufs=1) as statin:
            xbf = statin.tile([P, DT, LQ], BF16, tag="xbf")
            # LN0 stats accumulators (sum x / sum x^2 per query)
            sac1 = statin.tile([1, LQ], F32, tag="sac1")
            sac2 = statin.tile([1, LQ], F32, tag="sac2")

            with (
                tc.tile_pool(name="wts", bufs=1) as wts,
                tc.tile_pool(name="kvin", bufs=3) as kvin,
                tc.tile_pool(name="pT", bufs=6) as pTp,
                tc.tile_pool(name="x2p", bufs=2) as x2p,
                tc.tile_pool(name="bcsp", bufs=4) as bcsp,
            ):
                # PE warm-up (HAM clock gate)
                for w in range(12):
                    wps = ps.tile([P, 512], F32, tag="proj", bufs=1,
                                  name=f"warm{w}")
                    nc.tensor.matmul(wps[:, :], warm[:, 0:P], warm[:, :],
                                     start=True, stop=True)

                WqT_s = wts.tile([P, DT, D], BF16)
                WkT_s = wts.tile([P, DT, D], K8)
                WvT_s = wts.tile([P, DT, D], BF16)
                nc.sync.dma_start(out=WkT_s, in_=WkT[:, :, :])
                nc.sync.dma_start(out=WvT_s, in_=WvT[:, :, :])
                nc.sync.dma_start(out=WqT_s, in_=WqT[:, :, :])

                def emit_kvload(h):
                    t = h // 2
                    nk = kts[t] * P
                    kin = kvin.tile([P, DT, nkmax], K8, tag="kin",
                                    name=f"kin{h}")
                    vin = kvin.tile([P, DT, nkmax], V8, tag="vin",
                                    name=f"vin{h}")
                    nc.sync.dma_start(out=kin[:, :, 0:nk],
                                      in_=kTp[h, :, :, 0:nk])
                    nc.sync.dma_start(out=vin[:, :, 0:nk],
                                      in_=vTp[h, :, :, 0:nk])
                    return kin, vin

                def kproj_units(t, kin_e, kin_o):
                    """K^T for pair t: head h lands on partitions
                    (h%2)*64 .. +64 of KT_s."""
                    nk = kts[t] * P

                    def unit(h, kin, c0):
                        po = (h % 2) * 64
                        cw = min(512, nk - c0)
                        kp = ps.tile([P, 512], F32, tag="proj", bufs=1,
                                     name=f"kp{h}_{c0}")
                        for dt in range(DT):
                            nc.tensor.matmul(
                                kp[po:po + 64, 0:cw],
                                WkT_s[:, dt, h * 64:h * 64 + 64],
                                kin[:, dt, c0:c0 + cw],
                                start=(dt == 0), stop=(dt == DT - 1))
                        nc.vector.tensor_copy(
                            KT_s[po:po + 64, t, c0:c0 + cw],
                            kp[po:po + 64, 0:cw])

                    us = []
                    for h, kin in ((2 * t, kin_e), (2 * t + 1, kin_o)):
                        for c0 in range(0, nk, 512):
                            us.append(lambda h=h, kin=kin, c0=c0:
                                      unit(h, kin, c0))
                    return us

                def vproj_units(h, vin):
                    """V rows (packed keys on partitions) for head h."""
                    t = h // 2
                    kt_n = kts[t]

                    def unit(kt):
                        vp = ps.tile([P, 64], F32, tag="proj", bufs=1,
                                     name=f"vp{h}_{kt}")
                        for dt in range(DT):
                            nc.tensor.matmul(
                                vp[:, :],
                                vin[:, dt, kt * P:(kt + 1) * P],
                                WvT_s[:, dt, h * 64:h * 64 + 64],
                                start=(dt == 0), stop=(dt == DT - 1))
                        nc.vector.tensor_copy(V_s[:, h, kt, 0:DV], vp[:, :])

                    return [lambda kt=kt: unit(kt) for kt in range(kt_n)]

                def qproj_units(mt):
                    def unit(jb):
                        qp = ps.tile([P, 512], F32, tag="proj", bufs=1,
                                     name=f"qp{mt}_{jb}")
                        for dt in range(DT):
                            nc.tensor.matmul(
                                qp[:, :],
                                WqT_s[:, dt, mt * P:(mt + 1) * P],
                                qT_s[:, dt, jb * 512:(jb + 1) * 512],
                                start=(dt == 0), stop=(dt == DT - 1))
                        nc.vector.tensor_copy(
                            QT_s[:, mt, jb * 512:(jb + 1) * 512], qp[:, :])

                    return [lambda jb=jb: unit(jb) for jb in range(NB)]

                def emit_qk(t, kt):
                    """Row-tiled score matmul pair for (pair t, key tile kt).
                    Returns (pse, pso) PSUM tiles [128, LQ]."""
                    pse = ps.tile([P, LQ], F32, tag="qk", bufs=2,
                                  name=f"qe{t}_{kt}")
                    pso = ps.tile([P, LQ], F32, tag="qk", bufs=2,
                                  name=f"qo{t}_{kt}")
                    for jb in range(NB):
                        sl = slice(jb * 512, (jb + 1) * 512)
                        nc.tensor.matmul(
                            pse[:, sl], KT_s[0:64, t, kt * P:(kt + 1) * P],
                            QT_s[0:64, t, sl], start=True, stop=True)
                        nc.tensor.matmul(
                            pso[:, sl], KT_s[64:128, t, kt * P:(kt + 1) * P],
                            QT_s[64:128, t, sl], start=True, stop=True)
                    return pse, pso

                def emit_attn(t, units):
                    """Head pair (2t, 2t+1), channel tile t of xbf.  The
                    projection thunks in `units` are sprinkled between the
                    kt steps so the PE fills ACT-wait gaps."""
                    he, ho = 2 * t, 2 * t + 1
                    kt_n = kts[t]
                    avs = [ps.tile([P, 512], F32, tag="av", bufs=2,
                                   name=f"av{t}_{j}") for j in range(NB)]
                    # denominators: one PSUM bank, 4 col-tiled slots
                    # (query-half x head) at partitions 0/32/64/96
                    dn = ps.tile([97, 512], F32, tag="dn", bufs=1,
                                 name=f"dn{t}")
                    nunit = len(units)
                    udone = 0
                    for kt in range(kt_n):
                        pse, pso = emit_qk(t, kt)
                        pte = pTp.tile([P, LQ], BF16, tag="pT")
                        nc.scalar.activation(out=pte, in_=pse[:, :],
                                             func=AF.Exp, scale=1.0 / 8.0)
                        pto = pTp.tile([P, LQ], BF16, tag="pT")
                        nc.scalar.activation(out=pto, in_=pso[:, :],
                                             func=AF.Exp, scale=1.0 / 8.0)
                        st = (kt == 0)
                        sp = (kt == kt_n - 1)
                        for jb in range(NB):
                            sl = slice(jb * 512, (jb + 1) * 512)
                            nc.tensor.matmul(
                                avs[jb][0:64, :], V_s[:, he, kt, 0:DV],
                                pte[:, sl], start=st, stop=sp,
                                skip_group_check=True)
                            nc.tensor.matmul(
                                avs[jb][64:128, :], V_s[:, ho, kt, 0:DV],
                                pto[:, sl], start=st, stop=sp,
                                skip_group_check=True)
                            po = jb * 64
                            nc.tensor.matmul(
                                dn[po:po + 1, :],
                                V_s[:, he, kt, DV:DV + 1],
                                pte[:, sl], start=st, stop=sp,
                                tile_position=(0, po), skip_group_check=True)
                            nc.tensor.matmul(
                                dn[po + 32:po + 33, :],
                                V_s[:, ho, kt, DV:DV + 1],
                                pto[:, sl], start=st, stop=sp,
                                tile_position=(0, po + 32),
                                skip_group_check=True)
                        # sprinkle next-pair projection work
                        want = (kt + 1) * nunit // kt_n
                        while udone < want:
                            units[udone]()
                            udone += 1
                    for jb in range(NB):
                        sl = slice(jb * 512, (jb + 1) * 512)
                        po = jb * 64
                        dst = bcsp.tile([33, 512], F32, tag="dstg",
                                        name=f"ds{t}_{jb}")
                        nc.vector.tensor_copy(dst[0:1, :], dn[po:po + 1, :])
                        nc.vector.tensor_copy(dst[32:33, :],
                                              dn[po + 32:po + 33, :])
                        rcd = dramp.tile([2, 512], F32, tag="rcd",
                                         name=f"rcd{t}_{jb}")
                        nc.sync.dma_start(out=rcd[0:1, :], in_=dst[0:1, :])
                        nc.sync.dma_start(out=rcd[1:2, :], in_=dst[32:33, :])
                        bcs = bcsp.tile([P, 512], F32, tag="bcs")
                        nc.gpsimd.dma_start(
                            out=bcs[0:64, :],
                            in_=rcd[0:1, :].to_broadcast([64, 512]))
                        nc.gpsimd.dma_start(
                            out=bcs[64:128, :],
                            in_=rcd[1:2, :].to_broadcast([64, 512]))
                        nc.vector.reciprocal_approx_fast(out=bcs, in_=bcs)
                        nc.vector.tensor_mul(xbf[:, t, sl], avs[jb][:, :],
                                             bcs[:, :])
                    # channel tile t complete: residual, x^2, LN0 stats
                    nc.gpsimd.tensor_add(xbf[:, t, :], xbf[:, t, :],
                                         qT_s[:, t, :])
                    x2t = x2p.tile([P, LQ], BF16, tag="x2t")
                    nc.vector.tensor_mul(x2t, xbf[:, t, :], xbf[:, t, :])
                    for jb in range(NB):
                        sl = slice(jb * 512, (jb + 1) * 512)
                        # LN0 stats via gpsimd partition all-reduce: keeps
                        # the pair-boundary work off the PE/PSUM queues so
                        # the next pair's QK issues immediately.
                        pr1 = bcsp.tile([P, 512], F32, tag="prd",
                                        name=f"pr1_{t}_{jb}")
                        nc.gpsimd.partition_all_reduce(
                            pr1, xbf[:, t, sl], P, bass_isa.ReduceOp.add)
                        if t == 0:
                            nc.vector.tensor_copy(sac1[:, sl], pr1[0:1, :])
                        else:
                            nc.vector.tensor_add(sac1[:, sl], sac1[:, sl],
                                                 pr1[0:1, :])
                        pr2 = bcsp.tile([P, 512], F32, tag="prd",
                                        name=f"pr2_{t}_{jb}")
                        nc.gpsimd.partition_all_reduce(
                            pr2, x2t[:, sl], P, bass_isa.ReduceOp.add)
                        if t == 0:
                            nc.vector.tensor_copy(sac2[:, sl], pr2[0:1, :])
                        else:
                            nc.vector.tensor_add(sac2[:, sl], sac2[:, sl],
                                                 pr2[0:1, :])

                kv = {}
                kv[0] = emit_kvload(0)
                kv[1] = emit_kvload(1)
                for u in qproj_units(0):
                    u()
                for u in kproj_units(0, kv[0][0], kv[1][0]):
                    u()
                for u in vproj_units(0, kv[0][1]):
                    u()
                for u in vproj_units(1, kv[1][1]):
                    u()
                kv[2] = emit_kvload(2)
                kv[3] = emit_kvload(3)
                units1 = (qproj_units(1)
                          + kproj_units(1, kv[2][0], kv[3][0])
                          + vproj_units(2, kv[2][1])
                          + vproj_units(3, kv[3][1]))
                for u in units1:
                    u()
                emit_attn(0, [])
                kv[4] = emit_kvload(4)
                kv[5] = emit_kvload(5)
                units2 = (qproj_units(2)
                          + kproj_units(2, kv[4][0], kv[5][0])
                          + vproj_units(4, kv[4][1])
                          + vproj_units(5, kv[5][1]))
                for u in units2:
                    u()
                emit_attn(1, [])
                kv[6] = emit_kvload(6)
                kv[7] = emit_kvload(7)
                units3 = (qproj_units(3)
                          + kproj_units(3, kv[6][0], kv[7][0])
                          + vproj_units(6, kv[6][1])
                          + vproj_units(7, kv[7][1]))
                for u in units3:
                    u()
                emit_attn(2, [])
                emit_attn(3, [])

            # ============ phase C: LN0 -> fc -> LN1 (all ^T, bf16) ========
            with (
                tc.tile_pool(name="lnp", bufs=1) as lnp,
                tc.tile_pool(name="chain", bufs=2) as chain,
                tc.tile_pool(name="bcB", bufs=4) as bcB,
                tc.tile_pool(name="wfc", bufs=1) as wfc,
            ):
                outT = lnp.tile([P, DT, LQ], BF16)
                ybf = lnp.tile([P, DT, LQ], BF16)
                y2bf = lnp.tile([P, DT, LQ], BF16)

                fcwT_s = wfc.tile([P, DT, D], BF16)
                nc.sync.dma_start(out=fcwT_s, in_=fcwT[:, :, :])

                def ln_apply(xb, s1ap, s2ap, g_idx, b_idx, nh, label,
                             final_out=None):
                    """One 512-query half of a transposed LayerNorm, applied
                    in place on the bf16 tile xb, from SBUF stat rows."""
                    sl = slice(nh * 512, (nh + 1) * 512)
                    mu = chain.tile([1, 512], F32, tag="mu")
                    nc.vector.tensor_scalar_mul(mu, s1ap, 1.0 / D)
                    var = chain.tile([1, 512], F32, tag="var")
                    nc.vector.tensor_mul(var, mu, mu)
                    msq = chain.tile([1, 512], F32, tag="msq")
                    nc.vector.tensor_scalar_mul(msq, s2ap, 1.0 / D)
                    nc.vector.tensor_sub(var, msq, var)
                    nc.scalar.activation(out=var, in_=var, func=AF.Sqrt,
                                         bias=eps_t[0:1, :])
                    rstd = chain.tile([1, 512], F32, tag="rstd")
                    nc.vector.reciprocal_approx_fast(out=rstd, in_=var)
                    mrb = chain.tile([1, 2, 512], BF16, tag="mrb")
                    nc.vector.tensor_copy(mrb[:, 0, :], mu[0:1, :])
                    nc.vector.tensor_copy(mrb[:, 1, :], rstd[0:1, :])
                    mu_b = ps.tile([P, 512], F32, tag="av", bufs=2,
                                   name=f"mb{label}{nh}")
                    nc.tensor.matmul(mu_b[:, :], ones_r1[:, :], mrb[:, 0, :],
                                     start=True, stop=True)
                    rstd_b = ps.tile([P, 512], F32, tag="av", bufs=2,
                                     name=f"rb{label}{nh}")
                    nc.tensor.matmul(rstd_b[:, :], ones_r1[:, :],
                                     mrb[:, 1, :], start=True, stop=True)
                    mu_bb = bcB.tile([P, 512], BF16, tag="bc",
                                     name=f"mbb{label}{nh}")
                    nc.scalar.copy(mu_bb, mu_b[:, :])
                    rstd_bb = bcB.tile([P, 512], BF16, tag="bc",
                                       name=f"rbb{label}{nh}")
                    nc.scalar.copy(rstd_bb, rstd_b[:, :])
                    for kt in range(DT):
                        nc.vector.tensor_sub(xb[:, kt, sl], xb[:, kt, sl],
                                             mu_bb[:, :])
                        nc.vector.tensor_mul(xb[:, kt, sl], xb[:, kt, sl],
                                             rstd_bb[:, :])
                        dst = xb if final_out is None else final_out
                        nc.scalar.activation(
                            out=dst[:, kt, sl], in_=xb[:, kt, sl],
                            func=AF.Identity,
                            scale=gbT[:, g_idx, kt:kt + 1],
                            bias=gbT[:, b_idx, kt:kt + 1])

                # LN0 both halves (in place on xbf = LN0 output, bf16)
                for nh in range(NB):
                    sl = slice(nh * 512, (nh + 1) * 512)
                    ln_apply(xbf, sac1[0:1, sl], sac2[0:1, sl], 0, 1, nh,
                             "a")
                for nh in range(NB):
                    sl = slice(nh * 512, (nh + 1) * 512)
                    # fc for this half; y = fc + fc_b + LN0 residual (bf16)
                    s1b = ps.tile([1, 512], F32, tag="av", bufs=2,
                                  name=f"s1b{nh}")
                    s2b = ps.tile([1, 512], F32, tag="av", bufs=2,
                                  name=f"s2b{nh}")
                    for m in range(DT):
                        fps = ps.tile([P, 512], F32, tag="qk", bufs=2,
                                      name=f"fc{m}_{nh}")
                        for dt in range(DT):
                            nc.tensor.matmul(
                                fps[:, :],
                                fcwT_s[:, dt, m * P:(m + 1) * P],
                                xbf[:, dt, sl],
                                start=(dt == 0), stop=(dt == DT - 1))
                        nc.scalar.activation(
                            out=ybf[:, m, sl], in_=fps[:, :],
                            func=AF.Identity, bias=gbT[:, 4, m:m + 1])
                        nc.vector.tensor_add(ybf[:, m, sl], ybf[:, m, sl],
                                             xbf[:, m, sl])
                        nc.gpsimd.tensor_mul(y2bf[:, m, sl], ybf[:, m, sl],
                                             ybf[:, m, sl])
                        nc.tensor.matmul(s1b[:, :], ones_st[:, :],
                                         ybf[:, m, sl],
                                         start=(m == 0), stop=(m == DT - 1))
                        nc.tensor.matmul(s2b[:, :], ones_st[:, :],
                                         y2bf[:, m, sl],
                                         start=(m == 0), stop=(m == DT - 1))
                    # LN1 on this half
                    ln_apply(ybf, s1b[:, :], s2b[:, :], 2, 3, nh, "b",
                             final_out=outT)
                    nc.sync.dma_start(out=out[:, :, sl],
                                      in_=outT[:, :, sl])


def _build(kts):
    key = ("nc", kts)
    if key in _CACHE:
        return _CACHE[key]
    nc = bacc.Bacc(None, target_bir_lowering=False, debug=False)
    with tile.TileContext(nc) as tc:
        _emit(nc, tc, kts)
    nc.compile()
    _CACHE[key] = nc
    return nc


def _prep_in_maps(q, k, v, mask, Wq, Wk, Wv, fc_w, fc_b, g0, b0, g1, b1):
    q = np.asarray(q, np.float32)
    k = np.asarray(k, np.float32)
    v = np.asarray(v, np.float32)
    mask = np.asarray(mask)
    bf = mybir.dt.np(BF16)
    k8 = mybir.dt.np(K8)
    v8 = mybir.dt.np(V8)

    def ptile(a):
        # [n, m] -> transpose -> [m(=tiles*128), n] -> [128, tiles, n]
        t = np.asarray(a, np.float32).T
        return np.ascontiguousarray(
            t.reshape(DT, P, t.shape[1]).transpose(1, 0, 2))

    WqTh = ptile(Wq).astype(bf)
    WkTh = ptile(Wk).astype(k8)
    WvTh = ptile(Wv).astype(bf)
    fcwTh = ptile(fc_w).astype(bf)
    vecs = np.stack([np.asarray(x, np.float32).reshape(DT, P).T
                     for x in (g0, b0, g1, b1, fc_b)])
    vecs = np.ascontiguousarray(vecs.transpose(1, 0, 2))  # [P, 5, DT]

    # per (b, h) packed key indices and tile counts
    idx = [[np.nonzero(mask[h * B + b])[0] for h in range(H)]
           for b in range(B)]
    nk = [[len(idx[b][h]) for h in range(H)] for b in range(B)]
    kts = tuple(
        max(1, max(int(math.ceil(nk[b][h] / P))
                   for b in range(B) for h in (2 * t, 2 * t + 1)))
        for t in range(H // 2))
    ktmax = max(kts)
    nkmax = ktmax * P

    # per-batch packed K/V (shared by the two query-half cores)
    kvb = []
    for b in range(B):
        kTpb = np.zeros((H, P, DT, nkmax), k8)
        vTpb = np.zeros((H, P, DT, nkmax), v8)
        ones = np.zeros((P, H, ktmax), np.float32)
        for h in range(H):
            n = nk[b][h]
            kTpb[h, :, :, 0:n] = ptile(k[b][idx[b][h]]).astype(k8)
            vTpb[h, :, :, 0:n] = ptile(v[b][idx[b][h]]).astype(v8)
            fl = np.zeros(nkmax, np.float32)
            fl[0:n] = 1.0
            ones[:, h, :] = fl[0:ktmax * P].reshape(ktmax, P).T
        kvb.append((np.ascontiguousarray(kTpb), np.ascontiguousarray(vTpb),
                    np.ascontiguousarray(
                        ones.reshape(P, H * ktmax)).astype(bf)))

    in_maps = []
    for c in range(NCORES):
        b = c // 2
        r0 = (c % 2) * LQ
        qTb = ptile(q[b][r0:r0 + LQ]).astype(bf)
        in_maps.append({
            "qT": qTb, "kTp": kvb[b][0], "vTp": kvb[b][1],
            "onescol": kvb[b][2],
            "WqT": WqTh, "WkT": WkTh, "WvT": WvTh, "fcwT": fcwTh,
            "vecs": vecs,
        })
    return in_maps, kts


def kernel(q, k, v, mask, Wq, Wk, Wv, fc_w, fc_b, g0, b0, g1, b1):
    in_maps, kts = _prep_in_maps(q, k, v, mask, Wq, Wk, Wv, fc_w, fc_b,
                                 g0, b0, g1, b1)
    nc = _build(kts)
    res = run_bass_kernel_spmd(nc, in_maps, core_ids=list(range(NCORES)))
    outf = np.empty((B, L, D), np.float32)
    for c in range(NCORES):
        b = c // 2
        r0 = (c % 2) * LQ
        o = np.asarray(res.results[c]["out"], np.float32)  # [128, DT, LQ]
        outf[b, r0:r0 + LQ, :] = o.transpose(2, 1, 0).reshape(LQ, D)
    return outf
